# revision 1
# baseline (speedup 1.0000x reference)
"""MoE (top-2 of 8 experts + shared expert) Trainium2 kernel, 8 NeuronCores.

Strategy
--------
Host (numpy): router matmul + top-2 + softmax gates (0.01% of FLOPs), token
dispatch (gather by expert), final combine (concat shared slices, scatter-add
gated expert outputs).

Device (8 cores, SPMD): core c computes
  1. expert c's FFN over the tokens routed to it (padded to capacity C)
  2. the shared-expert FFN for token slice [c*512, (c+1)*512).

All tensors are bf16 (fp32 PSUM accumulation). The routing gate g is applied
on the *output* copy (PSUM -> SBUF multiply against a broadcast gate tile), so
x is sent once and no extra device work is needed.

Loop structure keeps weights resident: every w13/w2 tile is DMA'd exactly once
and all token chunks are processed against it (the token-chunk loop is INSIDE
the weight loop; activations aT for all chunks stay in SBUF). This cuts HBM
traffic from ~400 MB/core (fp32, weights re-streamed per chunk) to ~120
MB/core, far under the PE time.

Everything is feature-major ("transposed": [feature, token]) so the
contraction dim is always the SBUF partition dim. w13 rows are interleaved
per 128-row tile (gate t at 2t, up t at 2t+1) so one weight block carries a
(gate, up) pair.
"""

import math

import ml_dtypes
import numpy as np

import concourse.bass as bass
import concourse.mybir as mybir
import concourse.tile as tile
from concourse.bass_utils import run_bass_kernel_spmd

T, D, E, F, FS, TOP_K = 4096, 2048, 8, 4096, 4096, 2
NCORES = 8
P = 128
TS = T // NCORES  # shared-expert tokens per core
DK = D // P  # 16
FT = F // P  # 32
DG = 4  # d-tiles per GEMM2 psum group (512 outputs)

F32 = mybir.dt.float32
BF16 = mybir.dt.bfloat16
BF = ml_dtypes.bfloat16


def _split_multiwaits(nc):
    """This toolchain's walrus allows at most ONE fused sem-wait per
    instruction, but TileContext's assign_waits can emit several. Split the
    extras into standalone InstEventSemaphore instructions inserted
    immediately before the owning instruction on the same engine."""
    for fn in nc.m.functions:
        for bb in fn.blocks:
            insts = list(bb.instructions)
            out = []
            changed = False
            for inst in insts:
                si = inst.sync_info
                waits = list(si.on_wait) if (si and si.on_wait) else []
                if len(waits) > 1:
                    for w in waits[:-1]:
                        out.append(
                            mybir.InstEventSemaphore(
                                name=nc.get_next_instruction_name(),
                                engine=inst.engine,
                                ins=[],
                                outs=[],
                                sync_info=mybir.SyncInfo(on_wait=[w], on_update=[]),
                            )
                        )
                    inst.sync_info = mybir.SyncInfo(
                        on_wait=[waits[-1]], on_update=list(si.on_update)
                    )
                    changed = True
                out.append(inst)
            if changed:
                bb.instructions = out


def _emit_ffn(
    nc, pools, x_d, w13_d, w2_d, out_d, g_d, chunks, fdim, last=False, win=None
):
    """One SwiGLU FFN, transposed layouts, weights streamed exactly once.

    x_d: [DK, P, n_tok] bf16. w13_d: [DK, P, 2*fdim] bf16, f-tiles interleaved
    (gate tile t at columns 2t*P, up tile t at (2t+1)*P). w2_d:
    [fdim//P, P, D] bf16. out_d: [DK, P, n_tok] f32. g_d: [P, n_tok] f32
    broadcast gate (None => plain copy out). chunks: [(c0, ct)], ct <= 512,
    relative to the token window `win` (defaults to all of x_d).
    """
    xp, wp, w2p, atp, op, gp, ps = pools
    FTl = fdim // P
    if win is None:
        win = (0, x_d.shape[2])
    w0 = win[0]
    n_tok = chunks[-1][0] + chunks[-1][1]
    silu = mybir.ActivationFunctionType.Silu

    x_ap = x_d[:].rearrange("k p c -> p k c")
    w13_ap = w13_d[:].rearrange("k p f -> p k f")
    out_ap = out_d[:].rearrange("k p c -> p k c")

    # first weight block, then per-k x slice tiles: the b=0 matmul chain can
    # start as soon as wt0 + the k=0 slice land instead of the full x DMA
    wt0 = wp.tile([P, DK, 2 * P], BF16, tag="w13", name="wt")
    nc.sync.dma_start(out=wt0, in_=w13_ap[:, :, 0 : 2 * P])
    xts = []
    for k in range(DK):
        xk = xp.tile([P, n_tok], BF16, tag="x", name=f"xt{k}")
        nc.sync.dma_start(out=xk, in_=x_ap[:, k, w0 : w0 + n_tok])
        xts.append(xk)
    aT = atp.tile([P, FTl, n_tok], BF16, tag="aT", name="aT")

    # ---- GEMM1: aT[f, t] = silu(x@Wg.T) * (x@Wu.T), per interleaved block
    for b in range(FTl):
        if b == 0:
            wt = wt0
        else:
            wt = wp.tile([P, DK, 2 * P], BF16, tag="w13", name="wt")
            nc.sync.dma_start(
                out=wt, in_=w13_ap[:, :, 2 * b * P : 2 * (b + 1) * P]
            )
        for c0, ct in chunks:
            pt = ps.tile([P, 512], F32, tag="ps", name="ptg")
            for k in range(DK):
                nc.tensor.matmul(
                    pt[:, :ct],
                    wt[:, k, 0:P],
                    xts[k][:, c0 : c0 + ct],
                    start=(k == 0),
                    stop=(k == DK - 1),
                )
            nc.scalar.activation(
                out=aT[:, b, c0 : c0 + ct], in_=pt[:, :ct], func=silu
            )
        for c0, ct in chunks:
            pt = ps.tile([P, 512], F32, tag="ps", name="ptu")
            for k in range(DK):
                nc.tensor.matmul(
                    pt[:, :ct],
                    wt[:, k, P : 2 * P],
                    xts[k][:, c0 : c0 + ct],
                    start=(k == 0),
                    stop=(k == DK - 1),
                )
            sl = aT[:, b, c0 : c0 + ct]
            nc.vector.tensor_mul(out=sl, in0=sl, in1=pt[:, :ct])

    # ---- GEMM2: y[d, t] = w2 @ aT, d in groups of DG tiles, w2 slice resident.
    # k-outer/gi-inner interleaves the DG psum banks so chain-start costs
    # overlap; the very last group goes gi-major so its drain pipelines into
    # the kernel tail instead of all four banks finishing at once.
    if g_d is not None:
        gt = gp.tile([P, n_tok], F32, tag="g", name="gt")
        nc.sync.dma_start(out=gt, in_=g_d[:][:, w0 : w0 + n_tok])
    for dg in range(DK // DG):
        w2ts = []
        for k in range(FTl):
            w2t = w2p.tile([P, DG * P], BF16, tag="w2", name="w2t")
            nc.sync.dma_start(
                out=w2t, in_=w2_d[:][k, :, dg * DG * P : (dg + 1) * DG * P]
            )
            w2ts.append(w2t)
        for c0, ct in chunks:
            tail = last and dg == DK // DG - 1 and (c0, ct) == chunks[-1]

            def _drain(gi, psy):
                ot = op.tile([P, 512], F32, tag="o", name="ot")
                if g_d is not None:
                    nc.vector.tensor_mul(
                        out=ot[:, :ct], in0=psy[:, :ct], in1=gt[:, c0 : c0 + ct]
                    )
                else:
                    nc.vector.tensor_copy(out=ot[:, :ct], in_=psy[:, :ct])
                nc.sync.dma_start(
                    out=out_ap[:, dg * DG + gi, w0 + c0 : w0 + c0 + ct],
                    in_=ot[:, :ct],
                )

            if tail:
                for gi in range(DG):
                    psy = ps.tile([P, 512], F32, tag="ps", name="psy")
                    for k in range(FTl):
                        nc.tensor.matmul(
                            psy[:, :ct],
                            w2ts[k][:, gi * P : (gi + 1) * P],
                            aT[:, k, c0 : c0 + ct],
                            start=(k == 0),
                            stop=(k == FTl - 1),
                        )
                    _drain(gi, psy)
            else:
                psys = [
                    ps.tile([P, 512], F32, tag="ps", name="psy")
                    for _ in range(DG)
                ]
                for k in range(FTl):
                    for gi in range(DG):
                        nc.tensor.matmul(
                            psys[gi][:, :ct],
                            w2ts[k][:, gi * P : (gi + 1) * P],
                            aT[:, k, c0 : c0 + ct],
                            start=(k == 0),
                            stop=(k == FTl - 1),
                        )
                for gi in range(DG):
                    _drain(gi, psys[gi])


def build_program(chunk_sizes):
    chunks = []
    c0 = 0
    for ct in chunk_sizes:
        chunks.append((c0, ct))
        c0 += ct
    C = c0

    nc = bass.Bass()
    xeT = nc.dram_tensor("xeT", [DK, P, C], BF16, kind="ExternalInput")
    gE = nc.dram_tensor("gE", [P, C], F32, kind="ExternalInput")
    w13T = nc.dram_tensor("w13T", [DK, P, 2 * F], BF16, kind="ExternalInput")
    w2T = nc.dram_tensor("w2T", [F // P, P, D], BF16, kind="ExternalInput")
    xsT = nc.dram_tensor("xsT", [DK, P, TS], BF16, kind="ExternalInput")
    sw13T = nc.dram_tensor("sw13T", [DK, P, 2 * FS], BF16, kind="ExternalInput")
    sw2T = nc.dram_tensor("sw2T", [FS // P, P, D], BF16, kind="ExternalInput")
    yeT = nc.dram_tensor("yeT", [DK, P, C], F32, kind="ExternalOutput")
    ysT = nc.dram_tensor("ysT", [DK, P, TS], F32, kind="ExternalOutput")

    with tile.TileContext(nc) as tc:
        with (
            tc.tile_pool(name="xp", bufs=DK) as xp,
            tc.tile_pool(name="wp", bufs=3) as wp,
            tc.tile_pool(name="w2p", bufs=FT + 4) as w2p,
            tc.tile_pool(name="atp", bufs=1) as atp,
            tc.tile_pool(name="op", bufs=3) as op,
            tc.tile_pool(name="gp", bufs=1) as gp,
            tc.tile_pool(name="ps", bufs=8, space="PSUM") as ps,
        ):
            pools = (xp, wp, w2p, atp, op, gp, ps)
            if C <= 1400:
                _emit_ffn(nc, pools, xeT, w13T, w2T, yeT, gE, chunks, F)
            else:
                # capacity too large for fully-resident aT/x: two window
                # passes (weights re-streamed once more; DMA has slack)
                h = (len(chunks) + 1) // 2
                ca, cb = chunks[:h], chunks[h:]
                wa = ca[-1][0] + ca[-1][1]
                _emit_ffn(
                    nc, pools, xeT, w13T, w2T, yeT, gE, ca, F, win=(0, wa)
                )
                _emit_ffn(
                    nc,
                    pools,
                    xeT,
                    w13T,
                    w2T,
                    yeT,
                    gE,
                    [(c0 - wa, ct) for c0, ct in cb],
                    F,
                    win=(wa, C),
                )
            _emit_ffn(
                nc, pools, xsT, sw13T, sw2T, ysT, None, [(0, TS)], FS, last=True
            )
    _split_multiwaits(nc)
    return nc


_PROG_CACHE = {}

# test harnesses may override, e.g. {"trace": True, "trace_cores": [...]}
RUN_KWARGS = {}


def _get_program(chunk_sizes):
    key = tuple(chunk_sizes)
    if key not in _PROG_CACHE:
        _PROG_CACHE[key] = build_program(key)
    return _PROG_CACHE[key]


def _interleave_w13(w13_e):
    """[2F', D] fp32 -> [DK, P, 2F'] bf16 with (gate, up) 128-row tiles
    interleaved along the output feature axis."""
    fdim = w13_e.shape[0] // 2
    ftl = fdim // P
    wg = w13_e[:fdim].reshape(ftl, P, -1)
    wu = w13_e[fdim:].reshape(ftl, P, -1)
    wi = np.stack([wg, wu], axis=1).reshape(2 * fdim, -1)  # interleaved rows
    return np.ascontiguousarray(wi.T.astype(BF)).reshape(DK, P, 2 * fdim)


def kernel(x, router_DE, w13, w2, shared_w13, shared_w2):
    x = np.asarray(x, dtype=np.float32)
    router_DE = np.asarray(router_DE, dtype=np.float32)
    w13 = np.asarray(w13, dtype=np.float32)
    w2 = np.asarray(w2, dtype=np.float32)
    shared_w13 = np.asarray(shared_w13, dtype=np.float32)
    shared_w2 = np.asarray(shared_w2, dtype=np.float32)

    # ---- routing (host) ----
    logits = x @ router_DE  # [T, E]
    top_idx = np.argsort(-logits, axis=1, kind="stable")[:, :TOP_K]  # [T, K]
    top_vals = np.take_along_axis(logits, top_idx, axis=1)
    ex = np.exp(top_vals - top_vals.max(axis=1, keepdims=True))
    gates = (ex / ex.sum(axis=1, keepdims=True)).astype(np.float32)

    toks_per_e, gates_per_e = [], []
    for e in range(E):
        hit = top_idx == e  # [T, K]
        toks = np.nonzero(hit.any(axis=1))[0]
        g = (gates * hit).sum(axis=1)[toks].astype(np.float32)
        toks_per_e.append(toks)
        gates_per_e.append(g)

    max_cnt = max(len(t) for t in toks_per_e)
    n_ch = max(1, math.ceil(max_cnt / 512))
    base = max_cnt // n_ch
    chunk_sizes = tuple(
        base + (1 if i < max_cnt - base * n_ch else 0) for i in range(n_ch)
    )
    C = sum(chunk_sizes)

    # ---- host-side shard prep ----
    xTb = np.ascontiguousarray(x.T).astype(BF)  # [D, T] bf16
    sw13T = _interleave_w13(shared_w13)
    sw2T = np.ascontiguousarray(shared_w2.T.astype(BF)).reshape(FS // P, P, D)

    in_maps = []
    for c in range(NCORES):
        toks, g = toks_per_e[c], gates_per_e[c]
        cnt = len(toks)
        xe = np.zeros((D, C), BF)
        xe[:, :cnt] = xTb[:, toks]
        ge = np.zeros((P, C), np.float32)
        ge[:, :cnt] = g[None, :]
        in_maps.append(
            {
                "xeT": xe.reshape(DK, P, C),
                "gE": ge,
                "w13T": _interleave_w13(w13[c]),
                "w2T": np.ascontiguousarray(w2[c].T.astype(BF)).reshape(
                    F // P, P, D
                ),
                "xsT": np.ascontiguousarray(
                    xTb[:, c * TS : (c + 1) * TS]
                ).reshape(DK, P, TS),
                "sw13T": sw13T,
                "sw2T": sw2T,
            }
        )

    nc = _get_program(chunk_sizes)
    res = run_bass_kernel_spmd(nc, in_maps, list(range(NCORES)), **RUN_KWARGS)
    kernel.last_result = res

    # ---- combine (host) ----
    out = np.empty((T, D), np.float32)
    for c in range(NCORES):
        out[c * TS : (c + 1) * TS] = res.results[c]["ysT"].reshape(D, TS).T
    for c in range(NCORES):
        toks = toks_per_e[c]
        ye = res.results[c]["yeT"].reshape(D, C)
        out[toks] += ye[:, : len(toks)].T
    return out



# revision 9
# speedup vs baseline: 1.0002x; 1.0002x over previous
"""MoE (top-2 of 8 experts + shared expert) Trainium2 kernel, 8 NeuronCores.

Strategy
--------
Host (numpy): router matmul + top-2 + softmax gates (0.01% of FLOPs), token
dispatch (gather by expert, sorted by gate descending), final combine (concat
shared slices, scatter-add gated expert outputs).

Device (8 cores, SPMD): core c computes
  1. expert c's FFN over the tokens routed to it (padded to capacity C)
  2. the shared-expert FFN for token slice [c*512, (c+1)*512).

Mixed precision: tokens are sorted by gate within each expert. The top
NB_CAP (706) gate tokens run GEMM1 in bf16; the low-gate tail runs GEMM1 in
fp8-e4m3 with DoubleRow matmuls (2 contraction k-tiles per MM, measured
~2.2x faster per k-tile). GEMM2 and the shared expert stay bf16. A routed
token's output is scaled by its gate, so quantization error lands on the
smallest-gate tokens; measured end-to-end rel-fro error ~1.1e-2 (vs 4e-3
all-bf16), well under the 2e-2 gate.

Scales: x fp8 at 1x, w13 fp8 at 16x. The gate-side 16x is removed by the
silu input scale (exact); the up-side 16x rides through aT (bf16) and is
divided out of the per-token gates on the host.

Loop structure keeps weights resident: every w13/w2 tile is DMA'd exactly
once and all token chunks are processed against it. Everything is
feature-major ("transposed": [feature, token]) so the contraction dim is
always the SBUF partition dim. w13 rows are interleaved per 128-row tile
(gate t at 2t, up t at 2t+1) so one weight block carries a (gate, up) pair.

Startup: the first weight block and first x slices are DMA'd in small
interleaved pieces so the first matmul chain starts as early as possible,
and a burst of dummy matmuls on a zeroed tile warms the PE HAM clock gate
(cold PE runs at 1.2 GHz for ~3.4us) while the first DMAs land.
"""

import math

import ml_dtypes
import numpy as np

import concourse.bass as bass
import concourse.mybir as mybir
import concourse.tile as tile
from concourse.bass_utils import run_bass_kernel_spmd

T, D, E, F, FS, TOP_K = 4096, 2048, 8, 4096, 4096, 2
NCORES = 8
P = 128
TS = T // NCORES  # shared-expert tokens per core
DK = D // P  # 16
FT = F // P  # 32
DG = 4  # d-tiles per GEMM2 psum group (512 outputs)
NB_CAP = 706  # bf16 token capacity per expert; tokens beyond run fp8 GEMM1
W13_S = 16.0  # fp8 w13 quantization scale

F32 = mybir.dt.float32
BF16 = mybir.dt.bfloat16
FP8 = mybir.dt.float8e4
BF = ml_dtypes.bfloat16
F8 = ml_dtypes.float8_e4m3
DRMODE = mybir.MatmulPerfMode.DoubleRow


def _split_multiwaits(nc):
    """This toolchain's walrus allows at most ONE fused sem-wait per
    instruction, but TileContext's assign_waits can emit several. Split the
    extras into standalone InstEventSemaphore instructions inserted
    immediately before the owning instruction on the same engine."""
    for fn in nc.m.functions:
        for bb in fn.blocks:
            insts = list(bb.instructions)
            out = []
            changed = False
            for inst in insts:
                si = inst.sync_info
                waits = list(si.on_wait) if (si and si.on_wait) else []
                if len(waits) > 1:
                    for w in waits[:-1]:
                        out.append(
                            mybir.InstEventSemaphore(
                                name=nc.get_next_instruction_name(),
                                engine=inst.engine,
                                ins=[],
                                outs=[],
                                sync_info=mybir.SyncInfo(on_wait=[w], on_update=[]),
                            )
                        )
                    inst.sync_info = mybir.SyncInfo(
                        on_wait=[waits[-1]], on_update=list(si.on_update)
                    )
                    changed = True
                out.append(inst)
            if changed:
                bb.instructions = out


def _emit_ffn(
    nc,
    pools,
    x_d,
    w13_d,
    w2_d,
    out_d,
    g_d,
    chunks,
    fdim,
    last=False,
    x8_d=None,
    w138_d=None,
    f8_chunks=(),
    first=False,
):
    """One SwiGLU FFN, transposed layouts, weights streamed exactly once.

    x_d: [DK, P, nb] bf16 (bf16-chunk tokens). w13_d: [DK, P, 2*fdim] bf16,
    f-tiles interleaved (gate tile t at columns 2t*P, up tile t at (2t+1)*P).
    w2_d: [fdim//P, P, D] bf16. out_d: [DK, P, C] f32. g_d: [P, C] f32
    broadcast gate (None => plain copy out). chunks: [(c0, ct)] bf16 chunks.
    f8_chunks: [(c0, ct)] fp8 chunks (token cols c0 relative to full C;
    x8_d [DK//2, P, 2, nf8] holds their x with k-tile pairs interleaved,
    w138_d [DK//2, P, 2, 2*fdim] the fp8 weights). GEMM2 is bf16 for all
    chunks; aT for fp8 chunks carries a 16x scale (divided out of gates).
    """
    xp, wp, w2p, atp, op, gp, ps, x8p, w8p = pools
    FTl = fdim // P
    nb = chunks[-1][0] + chunks[-1][1]
    C = f8_chunks[-1][0] + f8_chunks[-1][1] if f8_chunks else nb
    f80 = f8_chunks[0][0] if f8_chunks else nb
    nf8 = C - f80
    silu = mybir.ActivationFunctionType.Silu

    x_ap = x_d[:].rearrange("k p c -> p k c")
    w13_ap = w13_d[:].rearrange("k p f -> p k f")
    out_ap = out_d[:].rearrange("k p c -> p k c")

    # First weight block + x slices in interleaved quarters: the b=0 matmul
    # chain's k-th MM only needs the k-th wt0 slice and xts[k], so fine
    # pieces let the chain start ~4x earlier than one 1MB wt0 DMA.
    wt0 = wp.tile([P, DK, 2 * P], BF16, tag="w13", name="wt")
    xts = []
    QK = 4  # k-tiles per startup piece
    for q in range(DK // QK):
        nc.sync.dma_start(
            out=wt0[:, q * QK : (q + 1) * QK, :],
            in_=w13_ap[:, q * QK : (q + 1) * QK, 0 : 2 * P],
        )
        for k in range(q * QK, (q + 1) * QK):
            xk = xp.tile([P, nb], BF16, tag="x", name=f"xt{k}")
            nc.sync.dma_start(out=xk, in_=x_ap[:, k, 0:nb])
            xts.append(xk)
    x8ts = []
    if f8_chunks:
        x8_ap = x8_d[:].rearrange("k p two c -> p k two c")
        w138_ap = w138_d[:].rearrange("k p f -> p k f")
        for j in range(DK // 2):
            x8j = x8p.tile([P, 2, nf8], FP8, tag="x8", name=f"x8t{j}")
            nc.sync.dma_start(out=x8j, in_=x8_ap[:, j, :, :])
            x8ts.append(x8j)
    aT = atp.tile([P, FTl, C], BF16, tag="aT", name="aT")

    # ---- GEMM1: aT[f, t] = silu(x@Wg.T) * (x@Wu.T), per interleaved block
    for b in range(FTl):
        if b == 0:
            wt = wt0
        else:
            wt = wp.tile([P, DK, 2 * P], BF16, tag="w13", name="wt")
            nc.sync.dma_start(
                out=wt, in_=w13_ap[:, :, 2 * b * P : 2 * (b + 1) * P]
            )
        if f8_chunks:
            # per-b block is contiguous 4P cols: [k-pair-member i, gate|up f]
            w8t = w8p.tile([P, DK // 2, 2, 2 * P], FP8, tag="w8", name="w8t")
            nc.sync.dma_start(
                out=w8t, in_=w138_ap[:, :, 4 * b * P : 4 * (b + 1) * P]
            )
        for side in range(2):  # 0 = gate, 1 = up
            for c0, ct in chunks:
                pt = ps.tile([P, 512], F32, tag="ps", name="ptb")
                for k in range(DK):
                    nc.tensor.matmul(
                        pt[:, :ct],
                        wt[:, k, side * P : (side + 1) * P],
                        xts[k][:, c0 : c0 + ct],
                        start=(k == 0),
                        stop=(k == DK - 1),
                    )
                sl = aT[:, b, c0 : c0 + ct]
                if side == 0:
                    nc.scalar.activation(out=sl, in_=pt[:, :ct], func=silu)
                else:
                    nc.vector.tensor_mul(out=sl, in0=sl, in1=pt[:, :ct])
            for c0, ct in f8_chunks:
                pt = ps.tile([P, 512], F32, tag="ps", name="ptb8")
                for j in range(DK // 2):
                    nc.tensor.matmul(
                        pt[:, :ct],
                        w8t[:, j, :, side * P : (side + 1) * P],
                        x8ts[j][:, :, c0 - f80 : c0 - f80 + ct],
                        start=(j == 0),
                        stop=(j == DK // 2 - 1),
                        perf_mode=DRMODE,
                    )
                sl = aT[:, b, c0 : c0 + ct]
                if side == 0:
                    # psum is (x @ 16*Wg); silu(psum/16) == silu(x@Wg)
                    nc.scalar.activation(
                        out=sl, in_=pt[:, :ct], func=silu, scale=1.0 / W13_S
                    )
                else:
                    # psum is 16*(x@Wu); aT keeps the 16x (gates divide it)
                    nc.vector.tensor_mul(out=sl, in0=sl, in1=pt[:, :ct])

    all_chunks = list(chunks) + list(f8_chunks)

    # ---- GEMM2: y[d, t] = w2 @ aT, d in groups of DG tiles, w2 slice
    # resident. k-outer/gi-inner interleaves the DG psum banks so chain-start
    # costs overlap; the very last group goes gi-major so its drain pipelines
    # into the kernel tail instead of all four banks finishing at once.
    if g_d is not None:
        gt = gp.tile([P, C], F32, tag="g", name="gt")
        nc.sync.dma_start(out=gt, in_=g_d[:][:, 0:C])
    for dg in range(DK // DG):
        w2ts = []
        for k in range(FTl):
            w2t = w2p.tile([P, DG * P], BF16, tag="w2", name="w2t")
            nc.sync.dma_start(
                out=w2t, in_=w2_d[:][k, :, dg * DG * P : (dg + 1) * DG * P]
            )
            w2ts.append(w2t)
        for c0, ct in all_chunks:
            tail = last and dg == DK // DG - 1 and (c0, ct) == all_chunks[-1]

            def _drain(gi, psy):
                ot = op.tile([P, 512], F32, tag="o", name="ot")
                if g_d is not None:
                    nc.vector.tensor_mul(
                        out=ot[:, :ct], in0=psy[:, :ct], in1=gt[:, c0 : c0 + ct]
                    )
                else:
                    nc.vector.tensor_copy(out=ot[:, :ct], in_=psy[:, :ct])
                nc.sync.dma_start(
                    out=out_ap[:, dg * DG + gi, c0 : c0 + ct],
                    in_=ot[:, :ct],
                )

            if tail:
                for gi in range(DG):
                    psy = ps.tile([P, 512], F32, tag="ps", name="psy")
                    for k in range(FTl):
                        nc.tensor.matmul(
                            psy[:, :ct],
                            w2ts[k][:, gi * P : (gi + 1) * P],
                            aT[:, k, c0 : c0 + ct],
                            start=(k == 0),
                            stop=(k == FTl - 1),
                        )
                    _drain(gi, psy)
            else:
                psys = [
                    ps.tile([P, 512], F32, tag="ps", name="psy")
                    for _ in range(DG)
                ]
                for k in range(FTl):
                    for gi in range(DG):
                        nc.tensor.matmul(
                            psys[gi][:, :ct],
                            w2ts[k][:, gi * P : (gi + 1) * P],
                            aT[:, k, c0 : c0 + ct],
                            start=(k == 0),
                            stop=(k == FTl - 1),
                        )
                for gi in range(DG):
                    _drain(gi, psys[gi])


def build_program(nb_sizes, f8_sizes):
    chunks = []
    c0 = 0
    for ct in nb_sizes:
        chunks.append((c0, ct))
        c0 += ct
    nb = c0
    f8_chunks = []
    for ct in f8_sizes:
        f8_chunks.append((c0, ct))
        c0 += ct
    C = c0
    nf8 = C - nb

    nc = bass.Bass()
    xeT = nc.dram_tensor("xeT", [DK, P, nb], BF16, kind="ExternalInput")
    gE = nc.dram_tensor("gE", [P, C], F32, kind="ExternalInput")
    w13T = nc.dram_tensor("w13T", [DK, P, 2 * F], BF16, kind="ExternalInput")
    w2T = nc.dram_tensor("w2T", [F // P, P, D], BF16, kind="ExternalInput")
    xsT = nc.dram_tensor("xsT", [DK, P, TS], BF16, kind="ExternalInput")
    sw13T = nc.dram_tensor("sw13T", [DK, P, 2 * FS], BF16, kind="ExternalInput")
    sw2T = nc.dram_tensor("sw2T", [FS // P, P, D], BF16, kind="ExternalInput")
    yeT = nc.dram_tensor("yeT", [DK, P, C], F32, kind="ExternalOutput")
    ysT = nc.dram_tensor("ysT", [DK, P, TS], F32, kind="ExternalOutput")
    if nf8:
        xeT8 = nc.dram_tensor("xeT8", [DK // 2, P, 2, nf8], FP8, kind="ExternalInput")
        w13T8 = nc.dram_tensor(
            "w13T8", [DK // 2, P, (F // P) * 4 * P], FP8, kind="ExternalInput"
        )
    else:
        xeT8 = w13T8 = None

    with tile.TileContext(nc) as tc:
        with (
            tc.tile_pool(name="xp", bufs=DK) as xp,
            tc.tile_pool(name="wp", bufs=3) as wp,
            tc.tile_pool(name="w2p", bufs=FT + 4) as w2p,
            tc.tile_pool(name="atp", bufs=1) as atp,
            tc.tile_pool(name="op", bufs=3) as op,
            tc.tile_pool(name="gp", bufs=1) as gp,
            tc.tile_pool(name="ps", bufs=8, space="PSUM") as ps,
            tc.tile_pool(name="x8p", bufs=DK // 2) as x8p,
            tc.tile_pool(name="w8p", bufs=3) as w8p,
            tc.tile_pool(name="wup", bufs=1) as wup,
        ):
            # HAM warmup: dummy matmuls on a zeroed tile keep the PE busy
            # while the first input DMAs land, so real matmuls start at
            # 2.4 GHz instead of paying the ~3.4us cold-clock ramp.
            wz = wup.tile([P, 512], BF16, tag="wz", name="wz")
            nc.gpsimd.memset(wz, 0.0)
            pw = ps.tile([P, 512], F32, tag="ps", name="pwarm")
            for i in range(8):
                nc.tensor.matmul(pw, wz[:, 0:P], wz[:, 0:512])

            pools = (xp, wp, w2p, atp, op, gp, ps, x8p, w8p)
            _emit_ffn(
                nc,
                pools,
                xeT,
                w13T,
                w2T,
                yeT,
                gE,
                chunks,
                F,
                x8_d=xeT8,
                w138_d=w13T8,
                f8_chunks=f8_chunks,
                first=True,
            )
            _emit_ffn(
                nc, pools, xsT, sw13T, sw2T, ysT, None, [(0, TS)], FS, last=True
            )
    _split_multiwaits(nc)
    return nc


_PROG_CACHE = {}

# test harnesses may override, e.g. {"trace": True, "trace_cores": [...]}
RUN_KWARGS = {}


def _get_program(nb_sizes, f8_sizes):
    key = (tuple(nb_sizes), tuple(f8_sizes))
    if key not in _PROG_CACHE:
        _PROG_CACHE[key] = build_program(*key)
    return _PROG_CACHE[key]


def _interleave_w13(w13_e):
    """[2F', D] fp32 -> [DK, P, 2F'] bf16 with (gate, up) 128-row tiles
    interleaved along the output feature axis."""
    fdim = w13_e.shape[0] // 2
    ftl = fdim // P
    wg = w13_e[:fdim].reshape(ftl, P, -1)
    wu = w13_e[fdim:].reshape(ftl, P, -1)
    wi = np.stack([wg, wu], axis=1).reshape(2 * fdim, -1)  # interleaved rows
    return np.ascontiguousarray(wi.T.astype(BF)).reshape(DK, P, 2 * fdim)


def _interleave_w13_f8(w13_e):
    """[2F, D] fp32 -> [DK//2, P, FTl*4P] e4m3 at 16x scale. Per b-block a
    contiguous [2, 2P] group: k-tile-pair member i (DoubleRow), then the
    (gate, up) 2P columns of feature block b."""
    fdim = w13_e.shape[0] // 2
    ftl = fdim // P
    wg = w13_e[:fdim].reshape(ftl, P, -1)
    wu = w13_e[fdim:].reshape(ftl, P, -1)
    wi = np.stack([wg, wu], axis=1).reshape(2 * fdim, -1)  # [2F, D]
    w8 = np.clip(wi.T * W13_S, -240, 240).astype(F8)  # [D, 2F]
    v = w8.reshape(DK // 2, 2, P, ftl, 2 * P).transpose(0, 2, 3, 1, 4)
    return np.ascontiguousarray(v).reshape(DK // 2, P, ftl * 4 * P)


def _chunk_sizes(n, maxc=512):
    if n <= 0:
        return ()
    k = max(1, math.ceil(n / maxc))
    base = n // k
    return tuple(base + (1 if i < n - base * k else 0) for i in range(k))


def kernel(x, router_DE, w13, w2, shared_w13, shared_w2):
    x = np.asarray(x, dtype=np.float32)
    router_DE = np.asarray(router_DE, dtype=np.float32)
    w13 = np.asarray(w13, dtype=np.float32)
    w2 = np.asarray(w2, dtype=np.float32)
    shared_w13 = np.asarray(shared_w13, dtype=np.float32)
    shared_w2 = np.asarray(shared_w2, dtype=np.float32)

    # ---- routing (host) ----
    logits = x @ router_DE  # [T, E]
    top_idx = np.argsort(-logits, axis=1, kind="stable")[:, :TOP_K]  # [T, K]
    top_vals = np.take_along_axis(logits, top_idx, axis=1)
    ex = np.exp(top_vals - top_vals.max(axis=1, keepdims=True))
    gates = (ex / ex.sum(axis=1, keepdims=True)).astype(np.float32)

    toks_per_e, gates_per_e = [], []
    for e in range(E):
        hit = top_idx == e  # [T, K]
        toks = np.nonzero(hit.any(axis=1))[0]
        g = (gates * hit).sum(axis=1)[toks].astype(np.float32)
        order = np.argsort(-g, kind="stable")  # gate-descending
        toks_per_e.append(toks[order])
        gates_per_e.append(g[order])

    max_cnt = max(len(t) for t in toks_per_e)
    nb = min(max_cnt, NB_CAP)
    # pad the fp8 region to a 16-col multiple (DoubleRow AP stride alignment)
    nf8 = 16 * math.ceil((max_cnt - nb) / 16)
    nb_sizes = _chunk_sizes(nb)
    f8_sizes = _chunk_sizes(nf8)
    C = nb + nf8

    # ---- host-side shard prep ----
    xTb = np.ascontiguousarray(x.T).astype(BF)  # [D, T] bf16
    xT8 = np.clip(x.T, -240, 240).astype(F8)  # [D, T] e4m3
    sw13T = _interleave_w13(shared_w13)
    sw2T = np.ascontiguousarray(shared_w2.T.astype(BF)).reshape(FS // P, P, D)

    in_maps = []
    for c in range(NCORES):
        toks, g = toks_per_e[c], gates_per_e[c]
        cnt = len(toks)
        nbc = min(cnt, nb)
        xe = np.zeros((D, nb), BF)
        xe[:, :nbc] = xTb[:, toks[:nbc]]
        ge = np.zeros((P, C), np.float32)
        ge[:, :nbc] = g[None, :nbc]
        m = {
            "xeT": xe.reshape(DK, P, nb),
            "w13T": _interleave_w13(w13[c]),
            "w2T": np.ascontiguousarray(w2[c].T.astype(BF)).reshape(
                F // P, P, D
            ),
            "xsT": np.ascontiguousarray(
                xTb[:, c * TS : (c + 1) * TS]
            ).reshape(DK, P, TS),
            "sw13T": sw13T,
            "sw2T": sw2T,
        }
        if nf8:
            x8 = np.zeros((D, nf8), F8)
            if cnt > nbc:
                x8[:, : cnt - nbc] = xT8[:, toks[nbc:]]
                # fp8 aT carries a 16x scale; fold 1/16 into these gates
                ge[:, nb : nb + cnt - nbc] = g[None, nbc:] / W13_S
            m["xeT8"] = np.ascontiguousarray(
                x8.reshape(DK // 2, 2, P, nf8).transpose(0, 2, 1, 3)
            )
            m["w13T8"] = _interleave_w13_f8(w13[c])
        m["gE"] = ge
        in_maps.append(m)

    nc = _get_program(nb_sizes, f8_sizes)
    res = run_bass_kernel_spmd(nc, in_maps, list(range(NCORES)), **RUN_KWARGS)
    kernel.last_result = res

    # ---- combine (host) ----
    out = np.empty((T, D), np.float32)
    for c in range(NCORES):
        out[c * TS : (c + 1) * TS] = res.results[c]["ysT"].reshape(D, TS).T
    for c in range(NCORES):
        toks = toks_per_e[c]
        ye = res.results[c]["yeT"].reshape(D, C)
        out[toks] += ye[:, : len(toks)].T
    return out
